# revision 7
# baseline (speedup 1.0000x reference)
"""Trainium2 Bass kernel for AIMv2FlashAttention2 (packed varlen attention).

Problem: hidden [8192, 1024] = 8 packed sequences x 1024 tokens, dim=1024,
16 heads x 64 head_dim. qkv proj + RoPE (rotate-half) + block-diagonal
softmax attention + out proj.

Strategy: pure data parallelism -- attention is block-diagonal per sequence,
so each of the 8 NeuronCores processes one full sequence locally with
replicated weights. Zero collectives.

v7b structure (PE kept dense; ScalarE = pure exp pacer):
  - startup: xt chunks on the sync DMA ring, wv/cos/sin/sel on the scalar
    ring (parallel HWDGE rings).
  - all PSUM producer tiles are full [128, 1024] 2-bank tiles cycling
    through one double-buffered pool (scores, v chunks, qk chunks, proj
    accumulators) -> one wide evacuation per tile, no 1-bank stalls.
  - v chunks 0,1 + q/k chunks 0,1 before attention; v 2-7 spliced into
    quad 0 ih0; qkv group g+1 spliced into quad g.
  - normalization fused into the PV evacuation: softmax sums staged to
    SBUF once per segment, broadcast via a one-hot K=128 matmul, recip
    (approx_fast) on the broadcast tile, single tensor_tensor multiply
    evacuates PSUM->outT. Spliced into the next segment's first steps.
  - RoPE runs on GpSimd (otherwise idle), freeing VectorE for PSUM
    evacuations.
  - proj half A (chunks 0-5) spliced into quad 3; norm(3,ih0) + proj half
    B tc0-3 spliced into quad 3 ih1; tail = norm(3,ih1) + proj B tc4-7.
"""

import numpy as np
import ml_dtypes

import concourse.bass as bass
import concourse.bacc as bacc
import concourse.mybir as mybir
import concourse.tile as tile
from concourse.bass import ts

F32 = mybir.dt.float32
F16 = mybir.dt.bfloat16

P = 128
L = 1024          # tokens per sequence / core
DIM = 1024
H = 16            # heads
D = 64            # head dim
NCORES = 8
LAG = 2           # PV trails QK by this many jc steps


def build_nc(dbg=False):
    nc = bacc.Bacc(None)

    xT = nc.declare_dram_parameter("xT", [DIM, L], F16, isOutput=False)
    wqk = nc.declare_dram_parameter("wqk", [16, P, DIM], F16, isOutput=False)
    wv = nc.declare_dram_parameter("wv", [8, P, DIM], F16, isOutput=False)
    wp = nc.declare_dram_parameter("wp", [8, P, DIM], F16, isOutput=False)
    cos4 = nc.declare_dram_parameter("cos4", [P, L], F16, isOutput=False)
    sin4 = nc.declare_dram_parameter("sin4", [P, L], F16, isOutput=False)
    # sel128[k, cpar, m] = 1.0 where k == 32*(2*cpar + m//64): K=128 one-hot
    # matmul replicating softmax-sum rows (at partitions 0/32/64/96) onto
    # the [128, 512] head-pair layout
    sel = nc.declare_dram_parameter("sel", [P, 2, P], F16, isOutput=False)
    out = nc.declare_dram_parameter("out", [L, DIM], F32, isOutput=True)

    Exp = mybir.ActivationFunctionType.Exp
    MUL = mybir.AluOpType.mult
    ADD = mybir.AluOpType.add
    SUB = mybir.AluOpType.subtract

    with tile.TileContext(nc) as tc:
        with (
            tc.tile_pool(name="consts", bufs=1) as consts,
            tc.tile_pool(name="qk", bufs=1) as qkpool,
            tc.tile_pool(name="vmat", bufs=1) as vpool,
            tc.tile_pool(name="outTp", bufs=1) as opool,
            tc.tile_pool(name="small", bufs=1) as small,
            tc.tile_pool(name="xt", bufs=1) as xtp,
            tc.tile_pool(name="wqks", bufs=3) as wqks,
            tc.tile_pool(name="ropetmp", bufs=8) as rtmp,
            tc.tile_pool(name="wmat", bufs=8) as wmat,
            tc.tile_pool(name="probs", bufs=6) as probs,
            tc.tile_pool(name="stag", bufs=2) as stag,
            tc.tile_pool(name="rrec", bufs=4) as rrec,
            tc.tile_pool(name="y", bufs=2) as ypool,
            tc.tile_pool(name="yacc", bufs=1) as yaccp,
            tc.tile_pool(name="psB", bufs=2, space="PSUM") as psB,
            tc.tile_pool(name="psS", bufs=4, space="PSUM") as psS,
        ):
            # ---- startup DMAs: xt on sync ring, weights on scalar ring ----
            xt_sb = xtp.tile([P, 8, L], F16, tag="xt")
            wv_t = []
            for dc in range(8):
                nc.sync.dma_start(xt_sb[:, dc, :], xT[ts(dc, P), :])
                w = wmat.tile([P, DIM], F16, tag="w", name=f"wv{dc}")
                nc.scalar.dma_start(w[:], wv[dc])
                wv_t.append(w)

            cos_sb = consts.tile([P, L], F16, tag="cos")
            sin_sb = consts.tile([P, L], F16, tag="sin")
            ones_c = consts.tile([P, 1], F16, tag="ones")
            sel_sb = small.tile([P, 2, P], F16, tag="sel")
            nc.scalar.dma_start(cos_sb[:], cos4[:])
            nc.scalar.dma_start(sin_sb[:], sin4[:])
            nc.scalar.dma_start(sel_sb[:], sel[:])
            nc.gpsimd.memset(ones_c[:], 1.0)

            q_sb = qkpool.tile([P, 8, L], F16, tag="q")
            k_sb = qkpool.tile([P, 8, L], F16, tag="k")
            v_sb = vpool.tile([P, 8, H, D], F16, tag="v")
            outT = opool.tile([P, 8, L], F16, tag="o")
            y_acc = yaccp.tile([P, 8, L], F16, tag="ya")

            def v_chunk(tc_):
                """Produce v for token chunk tc_: 16 MMs + one wide evac."""
                V = psB.tile([P, L], F32, tag="pb", name="Vt")
                for jh in (0, 1):
                    jsl = slice(512 * jh, 512 * jh + 512)
                    for dc in range(8):
                        nc.tensor.matmul(
                            V[:, jsl],
                            lhsT=xt_sb[:, dc, ts(tc_, P)],
                            rhs=wv_t[dc][:, jsl],
                            start=(dc == 0), stop=(dc == 7),
                        )
                nc.vector.tensor_copy(
                    v_sb[:, tc_, :, :],
                    V[:].rearrange("p (h d) -> p h d", d=D),
                )

            def v_rest():
                """Generator: v chunks 2-7, one yield per chunk."""
                for tc_ in range(2, 8):
                    v_chunk(tc_)
                    yield

            def qk_chunk_pair(c):
                """Generator producing q or k chunks (c, c+1): one [P, L]
                PSUM tile per chunk, one wide evacuation, RoPE on GpSimd."""
                ev = []
                for cc in (c, c + 1):
                    wt = wqks.tile([P, DIM], F16, tag="wqk")
                    nc.sync.dma_start(wt[:], wqk[cc])
                    S = psB.tile([P, L], F32, tag="pb", name="Sqk")
                    for th in (0, 1):
                        tsl = slice(512 * th, 512 * th + 512)
                        for dc in range(8):
                            nc.tensor.matmul(
                                S[:, tsl],
                                lhsT=wt[:, ts(dc, P)],
                                rhs=xt_sb[:, dc, tsl],
                                start=(dc == 0), stop=(dc == 7),
                            )
                        yield
                    e = rtmp.tile([P, L], F16, tag="rt")
                    nc.vector.tensor_copy(e[:], S[:])
                    ev.append(e)
                    yield
                U, Lp = ev
                tgt = q_sb if c < 8 else k_sb
                ci = c if c < 8 else c - 8
                t1 = rtmp.tile([P, L], F16, tag="rt")
                t2 = rtmp.tile([P, L], F16, tag="rt")
                # U' = U*cos - L*sin ; L' = L*cos + U*sin  (GpSimd)
                nc.gpsimd.tensor_tensor(tgt[:, ci, :], U[:], cos_sb[:], MUL)
                nc.gpsimd.tensor_tensor(t1[:], Lp[:], sin_sb[:], MUL)
                nc.gpsimd.tensor_tensor(
                    tgt[:, ci, :], tgt[:, ci, :], t1[:], SUB)
                yield
                nc.gpsimd.tensor_tensor(
                    tgt[:, ci + 1, :], Lp[:], cos_sb[:], MUL)
                nc.gpsimd.tensor_tensor(t2[:], U[:], sin_sb[:], MUL)
                nc.gpsimd.tensor_tensor(
                    tgt[:, ci + 1, :], tgt[:, ci + 1, :], t2[:], ADD)
                yield

            def qkv_feeder(g):
                yield from qk_chunk_pair(2 * g)       # q chunks 2g, 2g+1
                yield from qk_chunk_pair(8 + 2 * g)   # k chunks 2g, 2g+1

            def drain(feeder):
                if feeder is not None:
                    for _ in feeder:
                        pass

            def chain(*gens):
                for gg in gens:
                    yield from gg

            def norm_evac(g, ih, pvAB, pvCD, st):
                """Generator: normalize+evacuate segment (g, ih) given its
                staged sums tile st. Spliced into the NEXT segment."""
                isl = slice(512 * ih, 512 * ih + 512)
                rs = []
                for cc in (2 * g, 2 * g + 1):
                    Rs = psB.tile([P, 512], F32, tag="pb", name="Rs")
                    nc.tensor.matmul(
                        Rs[:], lhsT=sel_sb[:, cc % 2, :], rhs=st[:],
                        start=True, stop=True,
                    )
                    Rr = rrec.tile([P, 512], F32, tag="rr")
                    nc.vector.reciprocal_approx_fast(out=Rr[:], in_=Rs[:])
                    rs.append(Rr)
                yield
                for cc, Rr, pvt in ((2 * g, rs[0], pvAB),
                                    (2 * g + 1, rs[1], pvCD)):
                    nc.vector.tensor_tensor(
                        outT[:, cc, isl], pvt[:], Rr[:], MUL)
                yield

            def attention_quad(g, feeder=None):
                """Runs both ih segments; returns the last segment's
                pending norm_evac generator (to splice into what follows).
                `feeder` items are consumed one per step, after the
                previous segment's pending norm."""
                heads = [4 * g + j for j in range(4)]
                pending = None
                for ih in (0, 1):
                    isl = slice(512 * ih, 512 * ih + 512)
                    pvAB = psS.tile([P, 512], F32, tag="pvs", name="pvAB")
                    pvCD = psS.tile([P, 512], F32, tag="pvs", name="pvCD")
                    sum4 = psS.tile([P, 512], F32, tag="pvs", name="sum4")
                    prbs = {}
                    for step in range(8 + LAG):
                        jc = step
                        if jc < 8:
                            SAB = psB.tile([P, L], F32, tag="pb", name="SAB")
                            SCD = psB.tile([P, L], F32, tag="pb", name="SCD")
                            s_of = {0: (SAB, 0), 1: (SAB, 512),
                                    2: (SCD, 0), 3: (SCD, 512)}
                            for lo in (0, 1):   # up halves then lo halves
                                for j in range(4):
                                    St, co = s_of[j]
                                    psl = slice(32 * j, 32 * j + 32)
                                    nc.tensor.matmul(
                                        St[:, co:co + 512],
                                        lhsT=k_sb[psl, 2 * g + lo, ts(jc, P)],
                                        rhs=q_sb[psl, 2 * g + lo, isl],
                                        start=(lo == 0), stop=(lo == 1),
                                        tile_position=(32 * j, 0),
                                    )
                            prbAB = probs.tile([P, L], F16, tag="pr")
                            prbCD = probs.tile([P, L], F16, tag="pr")
                            nc.scalar.activation(prbAB[:], SAB[:], Exp,
                                                 scale=0.125)
                            nc.scalar.activation(prbCD[:], SCD[:], Exp,
                                                 scale=0.125)
                            prbs[jc] = (prbAB, prbCD)
                        pj = step - LAG
                        if pj >= 0:
                            prbAB, prbCD = prbs.pop(pj)
                            p_of = {0: (prbAB, 0), 1: (prbAB, 512),
                                    2: (prbCD, 0), 3: (prbCD, 512)}
                            for j in range(4):
                                prb, co = p_of[j]
                                pvt = pvAB if j < 2 else pvCD
                                ro = (j % 2) * D
                                nc.tensor.matmul(
                                    pvt[ro:ro + D, :],
                                    lhsT=v_sb[:, pj, heads[j], :],
                                    rhs=prb[:, co:co + 512],
                                    start=(pj == 0), stop=(pj == 7),
                                    tile_position=(0, ro),
                                    skip_group_check=True,
                                )
                            for j in range(4):
                                prb, co = p_of[j]
                                nc.tensor.matmul(
                                    sum4[32 * j:32 * j + 1, :],
                                    lhsT=ones_c[:],
                                    rhs=prb[:, co:co + 512],
                                    start=(pj == 0), stop=(pj == 7),
                                    tile_position=(0, 32 * j),
                                    skip_group_check=True,
                                )
                        if pending is not None:
                            if next(pending, _SENT) is _SENT:
                                pending = None
                        elif feeder is not None:
                            next(feeder, None)
                    # stage sums to SBUF (one wide copy; rows 0/32/64/96
                    # hold the 4 head sums, other rows ignored by sel128)
                    st = stag.tile([P, 512], F16, tag="st")
                    nc.vector.tensor_copy(st[:], sum4[:])
                    pending = norm_evac(g, ih, pvAB, pvCD, st)
                return pending

            wp_t = []

            def proj_half_a():
                """Generator: proj over chunks 0..5 -> SBUF accumulator."""
                for cc in range(8):
                    w = wmat.tile([P, DIM], F16, tag="w", name=f"wp{cc}")
                    nc.sync.dma_start(w[:], wp[cc])
                    wp_t.append(w)
                yield
                for tc_ in range(8):
                    Y = psB.tile([P, L], F32, tag="pb", name="Ya")
                    for eh in (0, 1):
                        esl = slice(512 * eh, 512 * eh + 512)
                        for cc in range(6):
                            nc.tensor.matmul(
                                Y[:, esl],
                                lhsT=outT[:, cc, ts(tc_, P)],
                                rhs=wp_t[cc][:, esl],
                                start=(cc == 0), stop=(cc == 5),
                            )
                        yield
                    nc.vector.tensor_copy(y_acc[:, tc_, :], Y[:])

            def proj_half_b(tcs):
                """Generator: proj chunks 6,7 + accumulate + out DMA."""
                for tc_ in tcs:
                    Y = psB.tile([P, L], F32, tag="pb", name="Yb")
                    for eh in (0, 1):
                        esl = slice(512 * eh, 512 * eh + 512)
                        for cc in (6, 7):
                            nc.tensor.matmul(
                                Y[:, esl],
                                lhsT=outT[:, cc, ts(tc_, P)],
                                rhs=wp_t[cc][:, esl],
                                start=(cc == 6), stop=(cc == 7),
                            )
                    ysb = ypool.tile([P, DIM], F32, tag="y2")
                    for eh in (0, 1):
                        esl = slice(512 * eh, 512 * eh + 512)
                        nc.vector.tensor_tensor(
                            ysb[:, esl], Y[:, esl], y_acc[:, tc_, esl],
                            ADD)
                        nc.sync.dma_start(out[ts(tc_, P), esl], ysb[:, esl])
                    yield

            # ---------------- pipeline ----------------
            # pre-attention: v chunks 0,1 + q/k chunks 0,1 (dense)
            f0 = chain(qk_chunk_pair(0), qk_chunk_pair(8))
            v_chunk(0)
            next(f0, None)
            v_chunk(1)
            next(f0, None)
            drain(f0)

            fq0 = chain(v_rest(), qkv_feeder(1))
            p0 = attention_quad(0, fq0)
            drain(fq0)
            f1 = chain(p0, qkv_feeder(2))
            p1 = attention_quad(1, f1)
            drain(f1)
            f2 = chain(p1, qkv_feeder(3))
            p2 = attention_quad(2, f2)
            drain(f2)
            f3 = chain(p2, proj_half_a(), proj_half_b(range(4)))
            p3 = attention_quad(3, f3)
            drain(p3)
            drain(f3)
            drain(proj_half_b(range(4, 8)))

    nc.compile()
    return nc


_SENT = object()


def _qk_perm():
    """Column permutation for q (or k) weights: chunk 2g = upper halves
    (d 0:32) of heads 4g..4g+3, chunk 2g+1 = lower halves."""
    perm = []
    for g in range(4):
        for d0 in (0, 32):
            for j in range(4):
                h = 4 * g + j
                perm.extend(h * D + d for d in range(d0, d0 + 32))
    return np.asarray(perm)


def prep_shards(hidden_states, cos, sin, w_qkv, b_qkv, w_proj, b_proj,
                cu_seqlens=None):
    """Build the per-core input maps (host-side, numpy)."""
    perm = _qk_perm()
    wq = w_qkv[:, :DIM][:, perm]
    wk = w_qkv[:, DIM:2 * DIM][:, perm]
    wqk_cols = np.concatenate([wq, wk], axis=1)            # [1024, 2048]
    # Wqk[c, dp, dc*128 + j] = wqk_cols[dc*128 + dp, c*128 + j]
    Wqk = np.ascontiguousarray(
        wqk_cols.reshape(8, P, 16, P).transpose(2, 1, 0, 3).reshape(16, P, DIM)
    ).astype(ml_dtypes.bfloat16)
    Wv = np.ascontiguousarray(
        w_qkv[:, 2 * DIM:].reshape(8, P, DIM)).astype(ml_dtypes.bfloat16)
    Wp = np.ascontiguousarray(
        w_proj.reshape(8, P, DIM)).astype(ml_dtypes.bfloat16)

    in_maps = []
    for i in range(NCORES):
        sl = slice(i * L, (i + 1) * L)
        xT = np.ascontiguousarray(
            hidden_states[sl].T).astype(ml_dtypes.bfloat16)
        cosT = cos[sl, :D // 2].T.astype(np.float32)       # [32, 1024]
        sinT = sin[sl, :D // 2].T.astype(np.float32)
        cos4 = np.ascontiguousarray(
            np.tile(cosT, (4, 1))).astype(ml_dtypes.bfloat16)
        sin4 = np.ascontiguousarray(
            np.tile(sinT, (4, 1))).astype(ml_dtypes.bfloat16)
        in_maps.append({
            "xT": xT, "wqk": Wqk, "wv": Wv, "wp": Wp,
            "cos4": cos4, "sin4": sin4, "sel": _sel_mat(),
        })
    return in_maps


def _sel_mat():
    sel = np.zeros((P, 2, P), ml_dtypes.bfloat16)
    for cpar in range(2):
        for m in range(P):
            sel[32 * (2 * cpar + m // D), cpar, m] = 1.0
    return sel


_NC_CACHE = {}


def kernel(hidden_states, cos, sin, w_qkv, b_qkv, w_proj, b_proj,
           cu_seqlens=None, **_unused):
    hidden_states = np.asarray(hidden_states)
    assert hidden_states.shape == (NCORES * L, DIM)

    from concourse.bass_utils import run_bass_kernel_spmd

    if "nc" not in _NC_CACHE:
        _NC_CACHE["nc"] = build_nc()
    nc = _NC_CACHE["nc"]

    in_maps = prep_shards(np.asarray(hidden_states), np.asarray(cos),
                          np.asarray(sin), np.asarray(w_qkv),
                          np.asarray(b_qkv), np.asarray(w_proj),
                          np.asarray(b_proj))
    res = run_bass_kernel_spmd(nc, in_maps, core_ids=list(range(NCORES)))
    out = np.concatenate([res.results[i]["out"] for i in range(NCORES)],
                         axis=0)
    return out.astype(np.float32)


# revision 8
# speedup vs baseline: 1.1045x; 1.1045x over previous
"""Trainium2 Bass kernel for AIMv2FlashAttention2 (packed varlen attention).

Problem: hidden [8192, 1024] = 8 packed sequences x 1024 tokens, dim=1024,
16 heads x 64 head_dim. qkv proj + RoPE (rotate-half) + block-diagonal
softmax attention + out proj.

Strategy: pure data parallelism -- attention is block-diagonal per sequence,
so each of the 8 NeuronCores processes one full sequence locally with
replicated weights. Zero collectives.

v7b structure (PE kept dense; ScalarE = pure exp pacer):
  - startup: xt chunks on the sync DMA ring, wv/cos/sin/sel on the scalar
    ring (parallel HWDGE rings).
  - all PSUM producer tiles are full [128, 1024] 2-bank tiles cycling
    through one double-buffered pool (scores, v chunks, qk chunks, proj
    accumulators) -> one wide evacuation per tile, no 1-bank stalls.
  - v chunks 0,1 + q/k chunks 0,1 before attention; v 2-7 spliced into
    quad 0 ih0; qkv group g+1 spliced into quad g.
  - normalization fused into the PV evacuation: softmax sums staged to
    SBUF once per segment, broadcast via a one-hot K=128 matmul, recip
    (approx_fast) on the broadcast tile, single tensor_tensor multiply
    evacuates PSUM->outT. Spliced into the next segment's first steps.
  - RoPE runs on GpSimd (otherwise idle), freeing VectorE for PSUM
    evacuations.
  - proj half A (chunks 0-5) spliced into quad 3; norm(3,ih0) + proj half
    B tc0-3 spliced into quad 3 ih1; tail = norm(3,ih1) + proj B tc4-7.
"""

import numpy as np
import ml_dtypes

import concourse.bass as bass
import concourse.bacc as bacc
import concourse.mybir as mybir
import concourse.tile as tile
from concourse.bass import ts

F32 = mybir.dt.float32
F16 = mybir.dt.bfloat16

P = 128
L = 1024          # tokens per sequence / core
DIM = 1024
H = 16            # heads
D = 64            # head dim
NCORES = 8
LAG = 2           # PV trails QK by this many jc steps


def build_nc(dbg=False):
    nc = bacc.Bacc(None)

    xT = nc.declare_dram_parameter("xT", [DIM, L], F16, isOutput=False)
    wqk = nc.declare_dram_parameter("wqk", [16, P, DIM], F16, isOutput=False)
    wv = nc.declare_dram_parameter("wv", [8, P, DIM], F16, isOutput=False)
    wp = nc.declare_dram_parameter("wp", [8, P, DIM], F16, isOutput=False)
    cos4 = nc.declare_dram_parameter("cos4", [P, L], F16, isOutput=False)
    sin4 = nc.declare_dram_parameter("sin4", [P, L], F16, isOutput=False)
    # sel128[k, cpar, m] = 1.0 where k == 32*(2*cpar + m//64): K=128 one-hot
    # matmul replicating softmax-sum rows (at partitions 0/32/64/96) onto
    # the [128, 512] head-pair layout
    sel = nc.declare_dram_parameter("sel", [P, 2, P], F16, isOutput=False)
    out = nc.declare_dram_parameter("out", [L, DIM], F32, isOutput=True)

    Exp = mybir.ActivationFunctionType.Exp
    MUL = mybir.AluOpType.mult
    ADD = mybir.AluOpType.add
    SUB = mybir.AluOpType.subtract

    with tile.TileContext(nc) as tc:
        with (
            tc.tile_pool(name="consts", bufs=1) as consts,
            tc.tile_pool(name="qk", bufs=1) as qkpool,
            tc.tile_pool(name="vmat", bufs=1) as vpool,
            tc.tile_pool(name="outTp", bufs=1) as opool,
            tc.tile_pool(name="small", bufs=1) as small,
            tc.tile_pool(name="xt", bufs=1) as xtp,
            tc.tile_pool(name="wqks", bufs=3) as wqks,
            tc.tile_pool(name="ropetmp", bufs=8) as rtmp,
            tc.tile_pool(name="wmat", bufs=8) as wmat,
            tc.tile_pool(name="probs", bufs=6) as probs,
            tc.tile_pool(name="stag", bufs=2) as stag,
            tc.tile_pool(name="rrec", bufs=4) as rrec,
            tc.tile_pool(name="y", bufs=2) as ypool,
            tc.tile_pool(name="yacc", bufs=1) as yaccp,
            tc.tile_pool(name="psB", bufs=2, space="PSUM") as psB,
            tc.tile_pool(name="psS", bufs=4, space="PSUM") as psS,
        ):
            # ---- startup DMAs: xt on sync ring, weights on scalar ring ----
            xt_sb = xtp.tile([P, 8, L], F16, tag="xt")
            wv_t = []
            for dc in range(8):
                nc.sync.dma_start(xt_sb[:, dc, :], xT[ts(dc, P), :])
                w = wmat.tile([P, DIM], F16, tag="w", name=f"wv{dc}")
                nc.scalar.dma_start(w[:], wv[dc])
                wv_t.append(w)

            cos_sb = consts.tile([P, L], F16, tag="cos")
            sin_sb = consts.tile([P, L], F16, tag="sin")
            ones_c = consts.tile([P, 1], F16, tag="ones")
            sel_sb = small.tile([P, 2, P], F16, tag="sel")
            nc.scalar.dma_start(cos_sb[:], cos4[:])
            nc.scalar.dma_start(sin_sb[:], sin4[:])
            nc.scalar.dma_start(sel_sb[:], sel[:])
            nc.gpsimd.memset(ones_c[:], 1.0)

            q_sb = qkpool.tile([P, 8, L], F16, tag="q")
            k_sb = qkpool.tile([P, 8, L], F16, tag="k")
            v_sb = vpool.tile([P, 8, H, D], F16, tag="v")
            outT = opool.tile([P, 8, L], F16, tag="o")
            y_acc = yaccp.tile([P, 8, L], F16, tag="ya")

            def v_chunk(tc_):
                """Produce v for token chunk tc_: 16 MMs + one wide evac."""
                V = psB.tile([P, L], F32, tag="pb", name="Vt")
                for jh in (0, 1):
                    jsl = slice(512 * jh, 512 * jh + 512)
                    for dc in range(8):
                        nc.tensor.matmul(
                            V[:, jsl],
                            lhsT=xt_sb[:, dc, ts(tc_, P)],
                            rhs=wv_t[dc][:, jsl],
                            start=(dc == 0), stop=(dc == 7),
                        )
                nc.vector.tensor_copy(
                    v_sb[:, tc_, :, :],
                    V[:].rearrange("p (h d) -> p h d", d=D),
                )

            def v_rest():
                """Generator: v chunks 2-7, one yield per chunk."""
                for tc_ in range(2, 8):
                    v_chunk(tc_)
                    yield

            def qk_chunk_pair(c):
                """Generator producing q or k chunks (c, c+1): one [P, L]
                PSUM tile per chunk, one wide evacuation, RoPE on GpSimd."""
                ev = []
                for cc in (c, c + 1):
                    wt = wqks.tile([P, DIM], F16, tag="wqk")
                    nc.sync.dma_start(wt[:], wqk[cc])
                    S = psB.tile([P, L], F32, tag="pb", name="Sqk")
                    for th in (0, 1):
                        tsl = slice(512 * th, 512 * th + 512)
                        for dc in range(8):
                            nc.tensor.matmul(
                                S[:, tsl],
                                lhsT=wt[:, ts(dc, P)],
                                rhs=xt_sb[:, dc, tsl],
                                start=(dc == 0), stop=(dc == 7),
                            )
                        yield
                    e = rtmp.tile([P, L], F16, tag="rt")
                    nc.vector.tensor_copy(e[:], S[:])
                    ev.append(e)
                    yield
                U, Lp = ev
                tgt = q_sb if c < 8 else k_sb
                ci = c if c < 8 else c - 8
                t1 = rtmp.tile([P, L], F16, tag="rt")
                t2 = rtmp.tile([P, L], F16, tag="rt")
                # U' = U*cos - L*sin ; L' = L*cos + U*sin
                nc.vector.tensor_tensor(tgt[:, ci, :], U[:], cos_sb[:], MUL)
                nc.vector.tensor_tensor(t1[:], Lp[:], sin_sb[:], MUL)
                nc.vector.tensor_tensor(
                    tgt[:, ci, :], tgt[:, ci, :], t1[:], SUB)
                yield
                nc.vector.tensor_tensor(
                    tgt[:, ci + 1, :], Lp[:], cos_sb[:], MUL)
                nc.vector.tensor_tensor(t2[:], U[:], sin_sb[:], MUL)
                nc.vector.tensor_tensor(
                    tgt[:, ci + 1, :], tgt[:, ci + 1, :], t2[:], ADD)
                yield

            def qkv_feeder(g):
                yield from qk_chunk_pair(2 * g)       # q chunks 2g, 2g+1
                yield from qk_chunk_pair(8 + 2 * g)   # k chunks 2g, 2g+1

            def drain(feeder):
                if feeder is not None:
                    for _ in feeder:
                        pass

            def chain(*gens):
                for gg in gens:
                    yield from gg

            def norm_evac(g, ih, pvAB, pvCD, st):
                """Generator: normalize+evacuate segment (g, ih) given its
                staged sums tile st. Spliced into the NEXT segment."""
                isl = slice(512 * ih, 512 * ih + 512)
                rs = []
                for cc in (2 * g, 2 * g + 1):
                    Rs = psB.tile([P, 512], F32, tag="pb", name="Rs")
                    nc.tensor.matmul(
                        Rs[:], lhsT=sel_sb[:, cc % 2, :], rhs=st[:],
                        start=True, stop=True,
                    )
                    Rr = rrec.tile([P, 512], F32, tag="rr")
                    nc.vector.reciprocal_approx_fast(out=Rr[:], in_=Rs[:])
                    rs.append(Rr)
                yield
                for cc, Rr, pvt in ((2 * g, rs[0], pvAB),
                                    (2 * g + 1, rs[1], pvCD)):
                    nc.vector.tensor_tensor(
                        outT[:, cc, isl], pvt[:], Rr[:], MUL)
                yield

            def attention_quad(g, feeder=None):
                """Runs both ih segments; returns the last segment's
                pending norm_evac generator (to splice into what follows).
                `feeder` items are consumed one per step, after the
                previous segment's pending norm."""
                heads = [4 * g + j for j in range(4)]
                pending = None
                for ih in (0, 1):
                    isl = slice(512 * ih, 512 * ih + 512)
                    pvAB = psS.tile([P, 512], F32, tag="pvs", name="pvAB")
                    pvCD = psS.tile([P, 512], F32, tag="pvs", name="pvCD")
                    sum4 = psS.tile([P, 512], F32, tag="pvs", name="sum4")
                    prbs = {}
                    for step in range(8 + LAG):
                        jc = step
                        if jc < 8:
                            SAB = psB.tile([P, L], F32, tag="pb", name="SAB")
                            SCD = psB.tile([P, L], F32, tag="pb", name="SCD")
                            s_of = {0: (SAB, 0), 1: (SAB, 512),
                                    2: (SCD, 0), 3: (SCD, 512)}
                            for lo in (0, 1):   # up halves then lo halves
                                for j in range(4):
                                    St, co = s_of[j]
                                    psl = slice(32 * j, 32 * j + 32)
                                    nc.tensor.matmul(
                                        St[:, co:co + 512],
                                        lhsT=k_sb[psl, 2 * g + lo, ts(jc, P)],
                                        rhs=q_sb[psl, 2 * g + lo, isl],
                                        start=(lo == 0), stop=(lo == 1),
                                        tile_position=(32 * j, 0),
                                    )
                            prbAB = probs.tile([P, L], F16, tag="pr")
                            prbCD = probs.tile([P, L], F16, tag="pr")
                            nc.scalar.activation(prbAB[:], SAB[:], Exp,
                                                 scale=0.125)
                            nc.scalar.activation(prbCD[:], SCD[:], Exp,
                                                 scale=0.125)
                            prbs[jc] = (prbAB, prbCD)
                        pj = step - LAG
                        if pj >= 0:
                            prbAB, prbCD = prbs.pop(pj)
                            p_of = {0: (prbAB, 0), 1: (prbAB, 512),
                                    2: (prbCD, 0), 3: (prbCD, 512)}
                            for j in range(4):
                                prb, co = p_of[j]
                                pvt = pvAB if j < 2 else pvCD
                                ro = (j % 2) * D
                                nc.tensor.matmul(
                                    pvt[ro:ro + D, :],
                                    lhsT=v_sb[:, pj, heads[j], :],
                                    rhs=prb[:, co:co + 512],
                                    start=(pj == 0), stop=(pj == 7),
                                    tile_position=(0, ro),
                                    skip_group_check=True,
                                )
                            for j in range(4):
                                prb, co = p_of[j]
                                nc.tensor.matmul(
                                    sum4[32 * j:32 * j + 1, :],
                                    lhsT=ones_c[:],
                                    rhs=prb[:, co:co + 512],
                                    start=(pj == 0), stop=(pj == 7),
                                    tile_position=(0, 32 * j),
                                    skip_group_check=True,
                                )
                        if pending is not None:
                            if next(pending, _SENT) is _SENT:
                                pending = None
                        elif feeder is not None:
                            next(feeder, None)
                    # stage sums to SBUF (one wide copy; rows 0/32/64/96
                    # hold the 4 head sums, other rows ignored by sel128)
                    st = stag.tile([P, 512], F16, tag="st")
                    nc.vector.tensor_copy(st[:], sum4[:])
                    pending = norm_evac(g, ih, pvAB, pvCD, st)
                return pending

            wp_t = []

            def proj_half_a():
                """Generator: proj over chunks 0..5 -> SBUF accumulator."""
                for cc in range(8):
                    w = wmat.tile([P, DIM], F16, tag="w", name=f"wp{cc}")
                    nc.sync.dma_start(w[:], wp[cc])
                    wp_t.append(w)
                yield
                for tc_ in range(8):
                    Y = psB.tile([P, L], F32, tag="pb", name="Ya")
                    for eh in (0, 1):
                        esl = slice(512 * eh, 512 * eh + 512)
                        for cc in range(6):
                            nc.tensor.matmul(
                                Y[:, esl],
                                lhsT=outT[:, cc, ts(tc_, P)],
                                rhs=wp_t[cc][:, esl],
                                start=(cc == 0), stop=(cc == 5),
                            )
                        yield
                    nc.vector.tensor_copy(y_acc[:, tc_, :], Y[:])

            def proj_half_b(tcs):
                """Generator: proj chunks 6,7 + accumulate + out DMA."""
                for tc_ in tcs:
                    Y = psB.tile([P, L], F32, tag="pb", name="Yb")
                    for eh in (0, 1):
                        esl = slice(512 * eh, 512 * eh + 512)
                        for cc in (6, 7):
                            nc.tensor.matmul(
                                Y[:, esl],
                                lhsT=outT[:, cc, ts(tc_, P)],
                                rhs=wp_t[cc][:, esl],
                                start=(cc == 6), stop=(cc == 7),
                            )
                    ysb = ypool.tile([P, DIM], F32, tag="y2")
                    for eh in (0, 1):
                        esl = slice(512 * eh, 512 * eh + 512)
                        nc.vector.tensor_tensor(
                            ysb[:, esl], Y[:, esl], y_acc[:, tc_, esl],
                            ADD)
                        nc.sync.dma_start(out[ts(tc_, P), esl], ysb[:, esl])
                    yield

            # ---------------- pipeline ----------------
            # pre-attention: v chunks 0,1 + q/k chunks 0,1 (dense)
            f0 = chain(qk_chunk_pair(0), qk_chunk_pair(8))
            v_chunk(0)
            next(f0, None)
            v_chunk(1)
            next(f0, None)
            drain(f0)

            fq0 = chain(v_rest(), qkv_feeder(1))
            p0 = attention_quad(0, fq0)
            drain(fq0)
            f1 = chain(p0, qkv_feeder(2))
            p1 = attention_quad(1, f1)
            drain(f1)
            f2 = chain(p1, qkv_feeder(3))
            p2 = attention_quad(2, f2)
            drain(f2)
            f3 = chain(p2, proj_half_a(), proj_half_b(range(4)))
            p3 = attention_quad(3, f3)
            drain(p3)
            drain(f3)
            drain(proj_half_b(range(4, 8)))

    nc.compile()
    return nc


_SENT = object()


def _qk_perm():
    """Column permutation for q (or k) weights: chunk 2g = upper halves
    (d 0:32) of heads 4g..4g+3, chunk 2g+1 = lower halves."""
    perm = []
    for g in range(4):
        for d0 in (0, 32):
            for j in range(4):
                h = 4 * g + j
                perm.extend(h * D + d for d in range(d0, d0 + 32))
    return np.asarray(perm)


def prep_shards(hidden_states, cos, sin, w_qkv, b_qkv, w_proj, b_proj,
                cu_seqlens=None):
    """Build the per-core input maps (host-side, numpy)."""
    perm = _qk_perm()
    wq = w_qkv[:, :DIM][:, perm]
    wk = w_qkv[:, DIM:2 * DIM][:, perm]
    wqk_cols = np.concatenate([wq, wk], axis=1)            # [1024, 2048]
    # Wqk[c, dp, dc*128 + j] = wqk_cols[dc*128 + dp, c*128 + j]
    Wqk = np.ascontiguousarray(
        wqk_cols.reshape(8, P, 16, P).transpose(2, 1, 0, 3).reshape(16, P, DIM)
    ).astype(ml_dtypes.bfloat16)
    Wv = np.ascontiguousarray(
        w_qkv[:, 2 * DIM:].reshape(8, P, DIM)).astype(ml_dtypes.bfloat16)
    Wp = np.ascontiguousarray(
        w_proj.reshape(8, P, DIM)).astype(ml_dtypes.bfloat16)

    in_maps = []
    for i in range(NCORES):
        sl = slice(i * L, (i + 1) * L)
        xT = np.ascontiguousarray(
            hidden_states[sl].T).astype(ml_dtypes.bfloat16)
        cosT = cos[sl, :D // 2].T.astype(np.float32)       # [32, 1024]
        sinT = sin[sl, :D // 2].T.astype(np.float32)
        cos4 = np.ascontiguousarray(
            np.tile(cosT, (4, 1))).astype(ml_dtypes.bfloat16)
        sin4 = np.ascontiguousarray(
            np.tile(sinT, (4, 1))).astype(ml_dtypes.bfloat16)
        in_maps.append({
            "xT": xT, "wqk": Wqk, "wv": Wv, "wp": Wp,
            "cos4": cos4, "sin4": sin4, "sel": _sel_mat(),
        })
    return in_maps


def _sel_mat():
    sel = np.zeros((P, 2, P), ml_dtypes.bfloat16)
    for cpar in range(2):
        for m in range(P):
            sel[32 * (2 * cpar + m // D), cpar, m] = 1.0
    return sel


_NC_CACHE = {}


def kernel(hidden_states, cos, sin, w_qkv, b_qkv, w_proj, b_proj,
           cu_seqlens=None, **_unused):
    hidden_states = np.asarray(hidden_states)
    assert hidden_states.shape == (NCORES * L, DIM)

    from concourse.bass_utils import run_bass_kernel_spmd

    if "nc" not in _NC_CACHE:
        _NC_CACHE["nc"] = build_nc()
    nc = _NC_CACHE["nc"]

    in_maps = prep_shards(np.asarray(hidden_states), np.asarray(cos),
                          np.asarray(sin), np.asarray(w_qkv),
                          np.asarray(b_qkv), np.asarray(w_proj),
                          np.asarray(b_proj))
    res = run_bass_kernel_spmd(nc, in_maps, core_ids=list(range(NCORES)))
    out = np.concatenate([res.results[i]["out"] for i in range(NCORES)],
                         axis=0)
    return out.astype(np.float32)


# revision 14
# speedup vs baseline: 1.1353x; 1.0279x over previous
"""Trainium2 Bass kernel for AIMv2FlashAttention2 (packed varlen attention).

Problem: hidden [8192, 1024] = 8 packed sequences x 1024 tokens, dim=1024,
16 heads x 64 head_dim. qkv proj + RoPE (rotate-half) + block-diagonal
softmax attention + out proj.

Strategy: pure data parallelism -- attention is block-diagonal per sequence,
so each of the 8 NeuronCores processes one full sequence locally with
replicated weights. Zero collectives.

v7b structure (PE kept dense; ScalarE = pure exp pacer):
  - startup: xt chunks on the sync DMA ring, wv/cos/sin/sel on the scalar
    ring (parallel HWDGE rings).
  - all PSUM producer tiles are full [128, 1024] 2-bank tiles cycling
    through one double-buffered pool (scores, v chunks, qk chunks, proj
    accumulators) -> one wide evacuation per tile, no 1-bank stalls.
  - v chunks 0,1 + q/k chunks 0,1 before attention; v 2-7 spliced into
    quad 0 ih0; qkv group g+1 spliced into quad g.
  - normalization fused into the PV evacuation: softmax sums staged to
    SBUF once per segment, broadcast via a one-hot K=128 matmul, recip
    (approx_fast) on the broadcast tile, single tensor_tensor multiply
    evacuates PSUM->outT. Spliced into the next segment's first steps.
  - RoPE runs on GpSimd (otherwise idle), freeing VectorE for PSUM
    evacuations.
  - proj half A (chunks 0-5) spliced into quad 3; norm(3,ih0) + proj half
    B tc0-3 spliced into quad 3 ih1; tail = norm(3,ih1) + proj B tc4-7.
"""

import numpy as np
import ml_dtypes

import concourse.bass as bass
import concourse.bacc as bacc
import concourse.mybir as mybir
import concourse.tile as tile
from concourse.bass import ts

F32 = mybir.dt.float32
F16 = mybir.dt.bfloat16

P = 128
L = 1024          # tokens per sequence / core
DIM = 1024
H = 16            # heads
D = 64            # head dim
NCORES = 8
LAG = 2           # PV trails QK by this many jc steps


def build_nc(dbg=False):
    nc = bacc.Bacc(None)

    xT = nc.declare_dram_parameter("xT", [DIM, L], F16, isOutput=False)
    wqk = nc.declare_dram_parameter("wqk", [16, P, DIM], F16, isOutput=False)
    wv = nc.declare_dram_parameter("wv", [8, P, DIM], F16, isOutput=False)
    wp = nc.declare_dram_parameter("wp", [8, P, DIM], F16, isOutput=False)
    cos4 = nc.declare_dram_parameter("cos4", [P, L], F16, isOutput=False)
    sin4 = nc.declare_dram_parameter("sin4", [P, L], F16, isOutput=False)
    # sel128[k, cpar, m] = 1.0 where k == 32*(2*cpar + m//64): K=128 one-hot
    # matmul replicating softmax-sum rows (at partitions 0/32/64/96) onto
    # the [128, 512] head-pair layout
    sel = nc.declare_dram_parameter("sel", [P, 2, P], F16, isOutput=False)
    out = nc.declare_dram_parameter("out", [L, DIM], F32, isOutput=True)

    Exp = mybir.ActivationFunctionType.Exp
    MUL = mybir.AluOpType.mult
    ADD = mybir.AluOpType.add
    SUB = mybir.AluOpType.subtract

    with tile.TileContext(nc) as tc:
        with (
            tc.tile_pool(name="consts", bufs=1) as consts,
            tc.tile_pool(name="qk", bufs=1) as qkpool,
            tc.tile_pool(name="vmat", bufs=1) as vpool,
            tc.tile_pool(name="outTp", bufs=1) as opool,
            tc.tile_pool(name="small", bufs=1) as small,
            tc.tile_pool(name="xt", bufs=1) as xtp,
            tc.tile_pool(name="wqks", bufs=3) as wqks,
            tc.tile_pool(name="ropetmp", bufs=8) as rtmp,
            tc.tile_pool(name="wmat", bufs=8) as wmat,
            tc.tile_pool(name="wvp", bufs=1) as wvp,
            tc.tile_pool(name="probs", bufs=6) as probs,
            tc.tile_pool(name="stag", bufs=2) as stag,
            tc.tile_pool(name="rrec", bufs=4) as rrec,
            tc.tile_pool(name="y", bufs=2) as ypool,
            tc.tile_pool(name="yacc", bufs=1) as yaccp,
            tc.tile_pool(name="psB", bufs=2, space="PSUM") as psB,
            tc.tile_pool(name="psS", bufs=4, space="PSUM") as psS,
        ):
            # ---- startup DMAs: xt on sync ring, weights on scalar ring ----
            xt_sb = xtp.tile([P, 8, L], F16, tag="xt")
            wv_t = []
            for dc in range(8):
                nc.sync.dma_start(xt_sb[:, dc, :], xT[ts(dc, P), :])
                w = wmat.tile([P, DIM], F16, tag="w", name=f"wv{dc}")
                nc.scalar.dma_start(w[:], wv[dc])
                wv_t.append(w)

            cos_sb = consts.tile([P, L], F16, tag="cos")
            sin_sb = consts.tile([P, L], F16, tag="sin")
            ones_c = consts.tile([P, 1], F16, tag="ones")
            sel_sb = small.tile([P, 2, P], F16, tag="sel")
            nc.scalar.dma_start(cos_sb[:], cos4[:])
            nc.scalar.dma_start(sin_sb[:], sin4[:])
            nc.scalar.dma_start(sel_sb[:], sel[:])
            nc.gpsimd.memset(ones_c[:], 1.0)

            q_sb = qkpool.tile([P, 8, L], F16, tag="q")
            k_sb = qkpool.tile([P, 8, L], F16, tag="k")
            v_sb = vpool.tile([P, 8, H, D], F16, tag="v")
            outT = opool.tile([P, 8, L], F16, tag="o")
            y_acc = yaccp.tile([P, 8, L], F16, tag="ya")

            def v_chunk(tc_):
                """Produce v for token chunk tc_: 16 MMs + one wide evac."""
                V = psB.tile([P, L], F32, tag="pb", name="Vt")
                for jh in (0, 1):
                    jsl = slice(512 * jh, 512 * jh + 512)
                    for dc in range(8):
                        nc.tensor.matmul(
                            V[:, jsl],
                            lhsT=xt_sb[:, dc, ts(tc_, P)],
                            rhs=wv_t[dc][:, jsl],
                            start=(dc == 0), stop=(dc == 7),
                        )
                nc.vector.tensor_copy(
                    v_sb[:, tc_, :, :],
                    V[:].rearrange("p (h d) -> p h d", d=D),
                )

            def v_rest():
                """Generator: v chunks 2-7, one yield per chunk."""
                for tc_ in range(2, 8):
                    v_chunk(tc_)
                    yield 3520

            def qk_chunk_pair(c):
                """Generator producing q or k chunks (c, c+1): one [P, L]
                PSUM tile per chunk, one wide evacuation, RoPE on GpSimd."""
                ev = []
                for cc in (c, c + 1):
                    wt = wqks.tile([P, DIM], F16, tag="wqk")
                    nc.sync.dma_start(wt[:], wqk[cc])
                    S = psB.tile([P, L], F32, tag="pb", name="Sqk")
                    for th in (0, 1):
                        tsl = slice(512 * th, 512 * th + 512)
                        for dc in range(8):
                            nc.tensor.matmul(
                                S[:, tsl],
                                lhsT=wt[:, ts(dc, P)],
                                rhs=xt_sb[:, dc, tsl],
                                start=(dc == 0), stop=(dc == 7),
                            )
                        yield 1760
                    e = rtmp.tile([P, L], F16, tag="rt")
                    nc.vector.tensor_copy(e[:], S[:])
                    ev.append(e)
                    yield 0
                U, Lp = ev
                tgt = q_sb if c < 8 else k_sb
                ci = c if c < 8 else c - 8
                t1 = rtmp.tile([P, L], F16, tag="rt")
                t2 = rtmp.tile([P, L], F16, tag="rt")
                # U' = U*cos - L*sin ; L' = L*cos + U*sin
                nc.vector.tensor_tensor(tgt[:, ci, :], U[:], cos_sb[:], MUL)
                nc.vector.tensor_tensor(t1[:], Lp[:], sin_sb[:], MUL)
                nc.vector.tensor_tensor(
                    tgt[:, ci, :], tgt[:, ci, :], t1[:], SUB)
                yield 0
                nc.vector.tensor_tensor(
                    tgt[:, ci + 1, :], Lp[:], cos_sb[:], MUL)
                nc.vector.tensor_tensor(t2[:], U[:], sin_sb[:], MUL)
                nc.vector.tensor_tensor(
                    tgt[:, ci + 1, :], tgt[:, ci + 1, :], t2[:], ADD)
                yield 0

            def qkv_feeder(g):
                yield from qk_chunk_pair(2 * g)       # q chunks 2g, 2g+1
                yield from qk_chunk_pair(8 + 2 * g)   # k chunks 2g, 2g+1

            def drain(feeder):
                if feeder is not None:
                    for _ in feeder:
                        pass

            def chain(*gens):
                for gg in gens:
                    yield from gg

            def norm_evac(g, ih, pvAB, pvCD, st):
                """Generator: normalize+evacuate segment (g, ih) given its
                staged sums tile st. Spliced into the NEXT segment."""
                isl = slice(512 * ih, 512 * ih + 512)
                rs = []
                for cc in (2 * g, 2 * g + 1):
                    Rs = psB.tile([P, 512], F32, tag="pb", name="Rs")
                    nc.tensor.matmul(
                        Rs[:], lhsT=sel_sb[:, cc % 2, :], rhs=st[:],
                        start=True, stop=True,
                    )
                    Rr = rrec.tile([P, 512], F32, tag="rr")
                    nc.vector.reciprocal_approx_fast(out=Rr[:], in_=Rs[:])
                    rs.append(Rr)
                yield 440
                for cc, Rr, pvt in ((2 * g, rs[0], pvAB),
                                    (2 * g + 1, rs[1], pvCD)):
                    nc.vector.tensor_tensor(
                        outT[:, cc, isl], pvt[:], Rr[:], MUL)
                yield 0

            def attention_quad(g, feeder=None, pending=None,
                               feeder_ih1=None):
                """Runs both ih segments; returns the last segment's
                pending norm_evac generator. A segment's psS tiles are
                allocated only AFTER the previous segment's norm reads
                are emitted (WAR on pool slots); feeder work is burned
                between pending items to keep PE dense meanwhile."""
                heads = [4 * g + j for j in range(4)]

                def burn(budget):
                    nonlocal feeder
                    while feeder is not None and budget > 0:
                        w = next(feeder, _SENT)
                        if w is _SENT:
                            feeder = None
                            break
                        budget -= w

                for ih in (0, 1):
                    while pending is not None:
                        burn(1500)
                        if next(pending, _SENT) is _SENT:
                            pending = None
                    if ih == 1 and feeder_ih1 is not None:
                        feeder = (chain(feeder_ih1, feeder)
                                  if feeder is not None else feeder_ih1)
                    isl = slice(512 * ih, 512 * ih + 512)
                    pvAB = psS.tile([P, 512], F32, tag="pvs", name="pvAB")
                    pvCD = psS.tile([P, 512], F32, tag="pvs", name="pvCD")
                    sum4 = psS.tile([P, 512], F32, tag="pvs", name="sum4")
                    prbs = {}
                    for step in range(8 + LAG):
                        burn(2400)
                        jc = step
                        if jc < 8:
                            SAB = psB.tile([P, L], F32, tag="pb", name="SAB")
                            SCD = psB.tile([P, L], F32, tag="pb", name="SCD")
                            s_of = {0: (SAB, 0), 1: (SAB, 512),
                                    2: (SCD, 0), 3: (SCD, 512)}
                            for lo in (0, 1):   # up halves then lo halves
                                for j in range(4):
                                    St, co = s_of[j]
                                    psl = slice(32 * j, 32 * j + 32)
                                    nc.tensor.matmul(
                                        St[:, co:co + 512],
                                        lhsT=k_sb[psl, 2 * g + lo, ts(jc, P)],
                                        rhs=q_sb[psl, 2 * g + lo, isl],
                                        start=(lo == 0), stop=(lo == 1),
                                        tile_position=(32 * j, 0),
                                    )
                            prbAB = probs.tile([P, L], F16, tag="pr")
                            prbCD = probs.tile([P, L], F16, tag="pr")
                            nc.scalar.activation(prbAB[:], SAB[:], Exp,
                                                 scale=0.125)
                            nc.scalar.activation(prbCD[:], SCD[:], Exp,
                                                 scale=0.125)
                            prbs[jc] = (prbAB, prbCD)
                        pj = step - LAG
                        if pj >= 0:
                            prbAB, prbCD = prbs.pop(pj)
                            p_of = {0: (prbAB, 0), 1: (prbAB, 512),
                                    2: (prbCD, 0), 3: (prbCD, 512)}
                            for j in range(4):
                                prb, co = p_of[j]
                                pvt = pvAB if j < 2 else pvCD
                                ro = (j % 2) * D
                                nc.tensor.matmul(
                                    pvt[ro:ro + D, :],
                                    lhsT=v_sb[:, pj, heads[j], :],
                                    rhs=prb[:, co:co + 512],
                                    start=(pj == 0), stop=(pj == 7),
                                    tile_position=(0, ro),
                                    skip_group_check=True,
                                )
                            for j in range(4):
                                prb, co = p_of[j]
                                nc.tensor.matmul(
                                    sum4[32 * j:32 * j + 1, :],
                                    lhsT=ones_c[:],
                                    rhs=prb[:, co:co + 512],
                                    start=(pj == 0), stop=(pj == 7),
                                    tile_position=(0, 32 * j),
                                    skip_group_check=True,
                                )
                    # stage sums to SBUF (one wide copy; rows 0/32/64/96
                    # hold the 4 head sums, other rows ignored by sel128)
                    st = stag.tile([P, 512], F16, tag="st")
                    nc.vector.tensor_copy(st[:], sum4[:])
                    pending = norm_evac(g, ih, pvAB, pvCD, st)
                return pending

            wp_t = []

            def proj_half_a():
                """Generator: proj over chunks 0..5 -> SBUF accumulator."""
                for cc in range(8):
                    w = wmat.tile([P, DIM], F16, tag="w", name=f"wp{cc}")
                    nc.sync.dma_start(w[:], wp[cc])
                    wp_t.append(w)
                yield 0
                for tc_ in range(8):
                    Y = psB.tile([P, L], F32, tag="pb", name="Ya")
                    for eh in (0, 1):
                        esl = slice(512 * eh, 512 * eh + 512)
                        for cc in range(6):
                            nc.tensor.matmul(
                                Y[:, esl],
                                lhsT=outT[:, cc, ts(tc_, P)],
                                rhs=wp_t[cc][:, esl],
                                start=(cc == 0), stop=(cc == 5),
                            )
                        yield 1320
                    nc.vector.tensor_copy(y_acc[:, tc_, :], Y[:])

            def proj_half_b(tcs):
                """Generator: proj chunks 6,7 + accumulate + out DMA."""
                for tc_ in tcs:
                    Y = psB.tile([P, L], F32, tag="pb", name="Yb")
                    for eh in (0, 1):
                        esl = slice(512 * eh, 512 * eh + 512)
                        for cc in (6, 7):
                            nc.tensor.matmul(
                                Y[:, esl],
                                lhsT=outT[:, cc, ts(tc_, P)],
                                rhs=wp_t[cc][:, esl],
                                start=(cc == 6), stop=(cc == 7),
                            )
                    ysb = ypool.tile([P, DIM], F32, tag="y2")
                    for eh in (0, 1):
                        esl = slice(512 * eh, 512 * eh + 512)
                        nc.vector.tensor_tensor(
                            ysb[:, esl], Y[:, esl], y_acc[:, tc_, esl],
                            ADD)
                        nc.sync.dma_start(out[ts(tc_, P), esl], ysb[:, esl])
                    yield 880

            # ---------------- pipeline ----------------
            # pre-attention: v chunks 0,1 + q/k chunks 0,1 (dense)
            f0 = chain(qk_chunk_pair(0), qk_chunk_pair(8))
            v_chunk(0)
            next(f0, None)
            v_chunk(1)
            next(f0, None)
            drain(f0)

            fq0 = chain(v_rest(), qkv_feeder(1))
            p0 = attention_quad(0, fq0)
            drain(fq0)
            f1 = qkv_feeder(2)
            p1 = attention_quad(1, f1, pending=p0)
            drain(f1)
            f2 = qkv_feeder(3)
            p2 = attention_quad(2, f2, pending=p1)
            drain(f2)
            f3 = proj_half_a()
            p3 = attention_quad(3, f3, pending=p2,
                                feeder_ih1=proj_half_b(range(4)))
            drain(p3)
            drain(f3)
            drain(proj_half_b(range(4, 8)))

    nc.compile()
    return nc


_SENT = object()


def _qk_perm():
    """Column permutation for q (or k) weights: chunk 2g = upper halves
    (d 0:32) of heads 4g..4g+3, chunk 2g+1 = lower halves."""
    perm = []
    for g in range(4):
        for d0 in (0, 32):
            for j in range(4):
                h = 4 * g + j
                perm.extend(h * D + d for d in range(d0, d0 + 32))
    return np.asarray(perm)


def prep_shards(hidden_states, cos, sin, w_qkv, b_qkv, w_proj, b_proj,
                cu_seqlens=None):
    """Build the per-core input maps (host-side, numpy)."""
    perm = _qk_perm()
    wq = w_qkv[:, :DIM][:, perm]
    wk = w_qkv[:, DIM:2 * DIM][:, perm]
    wqk_cols = np.concatenate([wq, wk], axis=1)            # [1024, 2048]
    # Wqk[c, dp, dc*128 + j] = wqk_cols[dc*128 + dp, c*128 + j]
    Wqk = np.ascontiguousarray(
        wqk_cols.reshape(8, P, 16, P).transpose(2, 1, 0, 3).reshape(16, P, DIM)
    ).astype(ml_dtypes.bfloat16)
    Wv = np.ascontiguousarray(
        w_qkv[:, 2 * DIM:].reshape(8, P, DIM)).astype(ml_dtypes.bfloat16)
    Wp = np.ascontiguousarray(
        w_proj.reshape(8, P, DIM)).astype(ml_dtypes.bfloat16)

    in_maps = []
    for i in range(NCORES):
        sl = slice(i * L, (i + 1) * L)
        xT = np.ascontiguousarray(
            hidden_states[sl].T).astype(ml_dtypes.bfloat16)
        cosT = cos[sl, :D // 2].T.astype(np.float32)       # [32, 1024]
        sinT = sin[sl, :D // 2].T.astype(np.float32)
        cos4 = np.ascontiguousarray(
            np.tile(cosT, (4, 1))).astype(ml_dtypes.bfloat16)
        sin4 = np.ascontiguousarray(
            np.tile(sinT, (4, 1))).astype(ml_dtypes.bfloat16)
        in_maps.append({
            "xT": xT, "wqk": Wqk, "wv": Wv, "wp": Wp,
            "cos4": cos4, "sin4": sin4, "sel": _sel_mat(),
        })
    return in_maps


def _sel_mat():
    sel = np.zeros((P, 2, P), ml_dtypes.bfloat16)
    for cpar in range(2):
        for m in range(P):
            sel[32 * (2 * cpar + m // D), cpar, m] = 1.0
    return sel


_NC_CACHE = {}


def kernel(hidden_states, cos, sin, w_qkv, b_qkv, w_proj, b_proj,
           cu_seqlens=None, **_unused):
    hidden_states = np.asarray(hidden_states)
    assert hidden_states.shape == (NCORES * L, DIM)

    from concourse.bass_utils import run_bass_kernel_spmd

    if "nc" not in _NC_CACHE:
        _NC_CACHE["nc"] = build_nc()
    nc = _NC_CACHE["nc"]

    in_maps = prep_shards(np.asarray(hidden_states), np.asarray(cos),
                          np.asarray(sin), np.asarray(w_qkv),
                          np.asarray(b_qkv), np.asarray(w_proj),
                          np.asarray(b_proj))
    res = run_bass_kernel_spmd(nc, in_maps, core_ids=list(range(NCORES)))
    out = np.concatenate([res.results[i]["out"] for i in range(NCORES)],
                         axis=0)
    return out.astype(np.float32)


# revision 15
# speedup vs baseline: 1.1465x; 1.0098x over previous
"""Trainium2 Bass kernel for AIMv2FlashAttention2 (packed varlen attention).

Problem: hidden [8192, 1024] = 8 packed sequences x 1024 tokens, dim=1024,
16 heads x 64 head_dim. qkv proj + RoPE (rotate-half) + block-diagonal
softmax attention + out proj.

Strategy: pure data parallelism -- attention is block-diagonal per sequence,
so each of the 8 NeuronCores processes one full sequence locally with
replicated weights. Zero collectives.

v7b structure (PE kept dense; ScalarE = pure exp pacer):
  - startup: xt chunks on the sync DMA ring, wv/cos/sin/sel on the scalar
    ring (parallel HWDGE rings).
  - all PSUM producer tiles are full [128, 1024] 2-bank tiles cycling
    through one double-buffered pool (scores, v chunks, qk chunks, proj
    accumulators) -> one wide evacuation per tile, no 1-bank stalls.
  - v chunks 0,1 + q/k chunks 0,1 before attention; v 2-7 spliced into
    quad 0 ih0; qkv group g+1 spliced into quad g.
  - normalization fused into the PV evacuation: softmax sums staged to
    SBUF once per segment, broadcast via a one-hot K=128 matmul, recip
    (approx_fast) on the broadcast tile, single tensor_tensor multiply
    evacuates PSUM->outT. Spliced into the next segment's first steps.
  - RoPE runs on GpSimd (otherwise idle), freeing VectorE for PSUM
    evacuations.
  - proj half A (chunks 0-5) spliced into quad 3; norm(3,ih0) + proj half
    B tc0-3 spliced into quad 3 ih1; tail = norm(3,ih1) + proj B tc4-7.
"""

import numpy as np
import ml_dtypes

import concourse.bass as bass
import concourse.bacc as bacc
import concourse.mybir as mybir
import concourse.tile as tile
from concourse.bass import ts

F32 = mybir.dt.float32
F16 = mybir.dt.bfloat16

P = 128
L = 1024          # tokens per sequence / core
DIM = 1024
H = 16            # heads
D = 64            # head dim
NCORES = 8
LAG = 2           # PV trails QK by this many jc steps


def build_nc(dbg=False):
    nc = bacc.Bacc(None)

    xT = nc.declare_dram_parameter("xT", [DIM, L], F16, isOutput=False)
    wqk = nc.declare_dram_parameter("wqk", [16, P, DIM], F16, isOutput=False)
    wv = nc.declare_dram_parameter("wv", [8, P, DIM], F16, isOutput=False)
    wp = nc.declare_dram_parameter("wp", [8, P, DIM], F16, isOutput=False)
    cos4 = nc.declare_dram_parameter("cos4", [P, L], F16, isOutput=False)
    sin4 = nc.declare_dram_parameter("sin4", [P, L], F16, isOutput=False)
    # sel128[k, cpar, m] = 1.0 where k == 32*(2*cpar + m//64): K=128 one-hot
    # matmul replicating softmax-sum rows (at partitions 0/32/64/96) onto
    # the [128, 512] head-pair layout
    sel = nc.declare_dram_parameter("sel", [P, 2, P], F16, isOutput=False)
    out = nc.declare_dram_parameter("out", [L, DIM], F32, isOutput=True)

    Exp = mybir.ActivationFunctionType.Exp
    MUL = mybir.AluOpType.mult
    ADD = mybir.AluOpType.add
    SUB = mybir.AluOpType.subtract

    with tile.TileContext(nc) as tc:
        with (
            tc.tile_pool(name="consts", bufs=1) as consts,
            tc.tile_pool(name="qk", bufs=1) as qkpool,
            tc.tile_pool(name="vmat", bufs=1) as vpool,
            tc.tile_pool(name="outTp", bufs=1) as opool,
            tc.tile_pool(name="small", bufs=1) as small,
            tc.tile_pool(name="xt", bufs=1) as xtp,
            tc.tile_pool(name="wqks", bufs=3) as wqks,
            tc.tile_pool(name="ropetmp", bufs=8) as rtmp,
            tc.tile_pool(name="wmat", bufs=8) as wmat,
            tc.tile_pool(name="wvp", bufs=1) as wvp,
            tc.tile_pool(name="probs", bufs=6) as probs,
            tc.tile_pool(name="stag", bufs=2) as stag,
            tc.tile_pool(name="rrec", bufs=4) as rrec,
            tc.tile_pool(name="y", bufs=2) as ypool,
            tc.tile_pool(name="yacc", bufs=1) as yaccp,
            tc.tile_pool(name="psB", bufs=2, space="PSUM") as psB,
            tc.tile_pool(name="psS", bufs=3, space="PSUM") as psS,
            tc.tile_pool(name="psF", bufs=1, space="PSUM") as psF,
        ):
            # ---- startup DMAs: xt on sync ring, weights on scalar ring ----
            xt_sb = xtp.tile([P, 8, L], F16, tag="xt")
            wv_t = []
            for dc in range(8):
                nc.sync.dma_start(xt_sb[:, dc, :], xT[ts(dc, P), :])
                w = wmat.tile([P, DIM], F16, tag="w", name=f"wv{dc}")
                nc.scalar.dma_start(w[:], wv[dc])
                wv_t.append(w)

            cos_sb = consts.tile([P, L], F16, tag="cos")
            sin_sb = consts.tile([P, L], F16, tag="sin")
            ones_c = consts.tile([P, 1], F16, tag="ones")
            sel_sb = small.tile([P, 2, P], F16, tag="sel")
            nc.scalar.dma_start(cos_sb[:], cos4[:])
            nc.scalar.dma_start(sin_sb[:], sin4[:])
            nc.scalar.dma_start(sel_sb[:], sel[:])
            nc.gpsimd.memset(ones_c[:], 1.0)

            q_sb = qkpool.tile([P, 8, L], F16, tag="q")
            k_sb = qkpool.tile([P, 8, L], F16, tag="k")
            v_sb = vpool.tile([P, 8, H, D], F16, tag="v")
            outT = opool.tile([P, 8, L], F16, tag="o")
            y_acc = yaccp.tile([P, 8, L], F16, tag="ya")

            def v_chunk_gen(tc_):
                """Generator: v for token chunk tc_ via psF halves."""
                for jh in (0, 1):
                    jsl = slice(512 * jh, 512 * jh + 512)
                    V = psF.tile([P, 512], F32, tag="pf", name="Vt")
                    for dc in range(8):
                        nc.tensor.matmul(
                            V[:],
                            lhsT=xt_sb[:, dc, ts(tc_, P)],
                            rhs=wv_t[dc][:, jsl],
                            start=(dc == 0), stop=(dc == 7),
                        )
                    nc.vector.tensor_copy(
                        v_sb[:, tc_, 8 * jh:8 * jh + 8, :],
                        V[:].rearrange("p (h d) -> p h d", d=D),
                    )
                    yield 1760

            def v_chunk(tc_):
                for _ in v_chunk_gen(tc_):
                    pass

            def v_rest():
                for tc_ in range(2, 8):
                    yield from v_chunk_gen(tc_)

            def qk_chunk_pair(c):
                """Generator producing q or k chunks (c, c+1): one [P, L]
                PSUM tile per chunk, one wide evacuation, RoPE on GpSimd."""
                ev = []
                for cc in (c, c + 1):
                    wt = wqks.tile([P, DIM], F16, tag="wqk")
                    nc.sync.dma_start(wt[:], wqk[cc])
                    e = rtmp.tile([P, L], F16, tag="rt")
                    for th in (0, 1):
                        tsl = slice(512 * th, 512 * th + 512)
                        S = psF.tile([P, 512], F32, tag="pf", name="Sqk")
                        for dc in range(8):
                            nc.tensor.matmul(
                                S[:],
                                lhsT=wt[:, ts(dc, P)],
                                rhs=xt_sb[:, dc, tsl],
                                start=(dc == 0), stop=(dc == 7),
                            )
                        nc.vector.tensor_copy(e[:, tsl], S[:])
                        yield 1760
                    ev.append(e)
                U, Lp = ev
                tgt = q_sb if c < 8 else k_sb
                ci = c if c < 8 else c - 8
                t1 = rtmp.tile([P, L], F16, tag="rt")
                t2 = rtmp.tile([P, L], F16, tag="rt")
                # U' = U*cos - L*sin ; L' = L*cos + U*sin
                nc.vector.tensor_tensor(tgt[:, ci, :], U[:], cos_sb[:], MUL)
                nc.vector.tensor_tensor(t1[:], Lp[:], sin_sb[:], MUL)
                nc.vector.tensor_tensor(
                    tgt[:, ci, :], tgt[:, ci, :], t1[:], SUB)
                yield 0
                nc.vector.tensor_tensor(
                    tgt[:, ci + 1, :], Lp[:], cos_sb[:], MUL)
                nc.vector.tensor_tensor(t2[:], U[:], sin_sb[:], MUL)
                nc.vector.tensor_tensor(
                    tgt[:, ci + 1, :], tgt[:, ci + 1, :], t2[:], ADD)
                yield 0

            def qkv_feeder(g):
                yield from qk_chunk_pair(2 * g)       # q chunks 2g, 2g+1
                yield from qk_chunk_pair(8 + 2 * g)   # k chunks 2g, 2g+1

            def drain(feeder):
                if feeder is not None:
                    for _ in feeder:
                        pass

            def chain(*gens):
                for gg in gens:
                    yield from gg

            def norm_evac(g, ih, pvAB, pvCD, st):
                """Generator: normalize+evacuate segment (g, ih) given its
                staged sums tile st. Spliced into the NEXT segment."""
                isl = slice(512 * ih, 512 * ih + 512)
                rs = []
                for cc in (2 * g, 2 * g + 1):
                    Rs = psF.tile([P, 512], F32, tag="pf", name="Rs")
                    nc.tensor.matmul(
                        Rs[:], lhsT=sel_sb[:, cc % 2, :], rhs=st[:],
                        start=True, stop=True,
                    )
                    Rr = rrec.tile([P, 512], F32, tag="rr")
                    nc.vector.reciprocal_approx_fast(out=Rr[:], in_=Rs[:])
                    rs.append(Rr)
                yield 440
                for cc, Rr, pvt in ((2 * g, rs[0], pvAB),
                                    (2 * g + 1, rs[1], pvCD)):
                    nc.vector.tensor_tensor(
                        outT[:, cc, isl], pvt[:], Rr[:], MUL)
                yield 0

            def attention_quad(g, feeder=None, pending=None,
                               feeder_ih1=None):
                """Runs both ih segments; returns the last segment's
                pending norm_evac generator. A segment's psS tiles are
                allocated only AFTER the previous segment's norm reads
                are emitted (WAR on pool slots); feeder work is burned
                between pending items to keep PE dense meanwhile."""
                heads = [4 * g + j for j in range(4)]

                def burn(budget):
                    nonlocal feeder
                    while feeder is not None and budget > 0:
                        w = next(feeder, _SENT)
                        if w is _SENT:
                            feeder = None
                            break
                        budget -= w

                for ih in (0, 1):
                    while pending is not None:
                        burn(1500)
                        if next(pending, _SENT) is _SENT:
                            pending = None
                    if ih == 1 and feeder_ih1 is not None:
                        feeder = (chain(feeder_ih1, feeder)
                                  if feeder is not None else feeder_ih1)
                    isl = slice(512 * ih, 512 * ih + 512)
                    pvAB = psS.tile([P, 512], F32, tag="pvs", name="pvAB")
                    pvCD = psS.tile([P, 512], F32, tag="pvs", name="pvCD")
                    sum4 = psS.tile([P, 512], F32, tag="pvs", name="sum4")
                    prbs = {}
                    for step in range(8 + LAG):
                        burn(1200)
                        jc = step
                        if jc < 8:
                            SAB = psB.tile([P, L], F32, tag="pb", name="SAB")
                            SCD = psB.tile([P, L], F32, tag="pb", name="SCD")
                            s_of = {0: (SAB, 0), 1: (SAB, 512),
                                    2: (SCD, 0), 3: (SCD, 512)}
                            for lo in (0, 1):   # up halves then lo halves
                                for j in range(4):
                                    St, co = s_of[j]
                                    psl = slice(32 * j, 32 * j + 32)
                                    nc.tensor.matmul(
                                        St[:, co:co + 512],
                                        lhsT=k_sb[psl, 2 * g + lo, ts(jc, P)],
                                        rhs=q_sb[psl, 2 * g + lo, isl],
                                        start=(lo == 0), stop=(lo == 1),
                                        tile_position=(32 * j, 0),
                                    )
                            prbAB = probs.tile([P, L], F16, tag="pr")
                            prbCD = probs.tile([P, L], F16, tag="pr")
                            nc.scalar.activation(prbAB[:], SAB[:], Exp,
                                                 scale=0.125)
                            nc.scalar.activation(prbCD[:], SCD[:], Exp,
                                                 scale=0.125)
                            prbs[jc] = (prbAB, prbCD)
                        burn(1200)
                        pj = step - LAG
                        if pj >= 0:
                            prbAB, prbCD = prbs.pop(pj)
                            p_of = {0: (prbAB, 0), 1: (prbAB, 512),
                                    2: (prbCD, 0), 3: (prbCD, 512)}
                            for j in range(4):
                                prb, co = p_of[j]
                                pvt = pvAB if j < 2 else pvCD
                                ro = (j % 2) * D
                                nc.tensor.matmul(
                                    pvt[ro:ro + D, :],
                                    lhsT=v_sb[:, pj, heads[j], :],
                                    rhs=prb[:, co:co + 512],
                                    start=(pj == 0), stop=(pj == 7),
                                    tile_position=(0, ro),
                                    skip_group_check=True,
                                )
                            for j in range(4):
                                prb, co = p_of[j]
                                nc.tensor.matmul(
                                    sum4[32 * j:32 * j + 1, :],
                                    lhsT=ones_c[:],
                                    rhs=prb[:, co:co + 512],
                                    start=(pj == 0), stop=(pj == 7),
                                    tile_position=(0, 32 * j),
                                    skip_group_check=True,
                                )
                    # stage sums to SBUF (one wide copy; rows 0/32/64/96
                    # hold the 4 head sums, other rows ignored by sel128)
                    st = stag.tile([P, 512], F16, tag="st")
                    nc.vector.tensor_copy(st[:], sum4[:])
                    pending = norm_evac(g, ih, pvAB, pvCD, st)
                return pending

            wp_t = []

            def proj_half_a():
                """Generator: proj over chunks 0..5 -> SBUF accumulator."""
                for cc in range(8):
                    w = wmat.tile([P, DIM], F16, tag="w", name=f"wp{cc}")
                    nc.sync.dma_start(w[:], wp[cc])
                    wp_t.append(w)
                yield 0
                for tc_ in range(8):
                    for eh in (0, 1):
                        esl = slice(512 * eh, 512 * eh + 512)
                        Y = psF.tile([P, 512], F32, tag="pf", name="Ya")
                        for cc in range(6):
                            nc.tensor.matmul(
                                Y[:],
                                lhsT=outT[:, cc, ts(tc_, P)],
                                rhs=wp_t[cc][:, esl],
                                start=(cc == 0), stop=(cc == 5),
                            )
                        nc.vector.tensor_copy(y_acc[:, tc_, esl], Y[:])
                        yield 1320

            def proj_half_b(tcs):
                """Generator: proj chunks 6,7 + accumulate + out DMA."""
                for tc_ in tcs:
                    ysb = ypool.tile([P, DIM], F32, tag="y2")
                    for eh in (0, 1):
                        esl = slice(512 * eh, 512 * eh + 512)
                        Y = psF.tile([P, 512], F32, tag="pf", name="Yb")
                        for cc in (6, 7):
                            nc.tensor.matmul(
                                Y[:],
                                lhsT=outT[:, cc, ts(tc_, P)],
                                rhs=wp_t[cc][:, esl],
                                start=(cc == 6), stop=(cc == 7),
                            )
                        nc.vector.tensor_tensor(
                            ysb[:, esl], Y[:], y_acc[:, tc_, esl], ADD)
                        nc.sync.dma_start(out[ts(tc_, P), esl], ysb[:, esl])
                        yield 440

            # ---------------- pipeline ----------------
            # pre-attention: v chunks 0,1 + q/k chunks 0,1 (dense)
            f0 = chain(qk_chunk_pair(0), qk_chunk_pair(8))
            v_chunk(0)
            next(f0, None)
            v_chunk(1)
            next(f0, None)
            drain(f0)

            fq0 = chain(v_rest(), qkv_feeder(1))
            p0 = attention_quad(0, fq0)
            drain(fq0)
            f1 = qkv_feeder(2)
            p1 = attention_quad(1, f1, pending=p0)
            drain(f1)
            f2 = qkv_feeder(3)
            p2 = attention_quad(2, f2, pending=p1)
            drain(f2)
            f3 = proj_half_a()
            p3 = attention_quad(3, f3, pending=p2,
                                feeder_ih1=proj_half_b(range(4)))
            drain(p3)
            drain(f3)
            drain(proj_half_b(range(4, 8)))

    nc.compile()
    return nc


_SENT = object()


def _qk_perm():
    """Column permutation for q (or k) weights: chunk 2g = upper halves
    (d 0:32) of heads 4g..4g+3, chunk 2g+1 = lower halves."""
    perm = []
    for g in range(4):
        for d0 in (0, 32):
            for j in range(4):
                h = 4 * g + j
                perm.extend(h * D + d for d in range(d0, d0 + 32))
    return np.asarray(perm)


def prep_shards(hidden_states, cos, sin, w_qkv, b_qkv, w_proj, b_proj,
                cu_seqlens=None):
    """Build the per-core input maps (host-side, numpy)."""
    perm = _qk_perm()
    wq = w_qkv[:, :DIM][:, perm]
    wk = w_qkv[:, DIM:2 * DIM][:, perm]
    wqk_cols = np.concatenate([wq, wk], axis=1)            # [1024, 2048]
    # Wqk[c, dp, dc*128 + j] = wqk_cols[dc*128 + dp, c*128 + j]
    Wqk = np.ascontiguousarray(
        wqk_cols.reshape(8, P, 16, P).transpose(2, 1, 0, 3).reshape(16, P, DIM)
    ).astype(ml_dtypes.bfloat16)
    Wv = np.ascontiguousarray(
        w_qkv[:, 2 * DIM:].reshape(8, P, DIM)).astype(ml_dtypes.bfloat16)
    Wp = np.ascontiguousarray(
        w_proj.reshape(8, P, DIM)).astype(ml_dtypes.bfloat16)

    in_maps = []
    for i in range(NCORES):
        sl = slice(i * L, (i + 1) * L)
        xT = np.ascontiguousarray(
            hidden_states[sl].T).astype(ml_dtypes.bfloat16)
        cosT = cos[sl, :D // 2].T.astype(np.float32)       # [32, 1024]
        sinT = sin[sl, :D // 2].T.astype(np.float32)
        cos4 = np.ascontiguousarray(
            np.tile(cosT, (4, 1))).astype(ml_dtypes.bfloat16)
        sin4 = np.ascontiguousarray(
            np.tile(sinT, (4, 1))).astype(ml_dtypes.bfloat16)
        in_maps.append({
            "xT": xT, "wqk": Wqk, "wv": Wv, "wp": Wp,
            "cos4": cos4, "sin4": sin4, "sel": _sel_mat(),
        })
    return in_maps


def _sel_mat():
    sel = np.zeros((P, 2, P), ml_dtypes.bfloat16)
    for cpar in range(2):
        for m in range(P):
            sel[32 * (2 * cpar + m // D), cpar, m] = 1.0
    return sel


_NC_CACHE = {}


def kernel(hidden_states, cos, sin, w_qkv, b_qkv, w_proj, b_proj,
           cu_seqlens=None, **_unused):
    hidden_states = np.asarray(hidden_states)
    assert hidden_states.shape == (NCORES * L, DIM)

    from concourse.bass_utils import run_bass_kernel_spmd

    if "nc" not in _NC_CACHE:
        _NC_CACHE["nc"] = build_nc()
    nc = _NC_CACHE["nc"]

    in_maps = prep_shards(np.asarray(hidden_states), np.asarray(cos),
                          np.asarray(sin), np.asarray(w_qkv),
                          np.asarray(b_qkv), np.asarray(w_proj),
                          np.asarray(b_proj))
    res = run_bass_kernel_spmd(nc, in_maps, core_ids=list(range(NCORES)))
    out = np.concatenate([res.results[i]["out"] for i in range(NCORES)],
                         axis=0)
    return out.astype(np.float32)
